# revision 1
# baseline (speedup 1.0000x reference)
"""CASSI shear kernel for Trainium2 (Bass/Tile), 8-core SPMD.

Computes, for full inputs x (1, 1024, 1024, 31) and ca (1, 1024, 1024, 1):
    y1[m, n, l] = x[m, n, l] * ca[m, n]
    out[m, j]   = sum_{n+l=j} y1[m, n, l]       (j in [0, 1054))
returning (1, 1024, 1054, 1) float32.

Sharding: rows m across 8 cores (128 rows/core = one full SBUF partition
block). Per core, free dim holds the (n, l) plane contiguously (n-major,
matching HBM layout so DMA loads are fully contiguous per partition).

Engine assignment (all three used concurrently, DMA-bound overall):
  - DMA: 8 x 2MB chunk loads (~50us/core total = the HBM roofline).
  - Vector engine: broadcast multiply y1 = x * ca in place (contiguous
    reads/writes, zero-stride broadcast of ca along l): ~34us.
  - Tensor engine: the 31-way shear scatter-add as identity-weight
    matmuls accumulating into PSUM: psum[:, n0+l : n0+l+C] += y1[:, :, l]
    (matmul with lhsT = I is a partition-preserving copy, and PSUM
    accumulation is free). One matmul per (chunk, l), split at PSUM bank
    boundaries; has_written is reset by 3 start=True zero-weight matmuls.
  - Scalar engine: evacuates PSUM -> SBUF at the end (DMA cannot read
    PSUM).
"""

import sys

import numpy as np

if "/opt/trn_rl_repo" not in sys.path:
    sys.path.insert(0, "/opt/trn_rl_repo")

M, N, L = 1024, 1024, 31
ONC = N + L - 1  # 1054
NCORES = 8
R = M // NCORES  # 128 rows per core
CHUNK = 256  # >= 256 so shear matmuls hit the f32r 1-cycle/row fast path
BANK = 512  # PSUM bank size in fp32 elements

_cached_nc = {}


def _shear_pieces(chunk, bf16mode):
    """All shear matmuls as (chunk_idx, l0, g, lstep, delta, t0, w, stop).

    Each matmul handles a GROUP of g l-values {l0, l0+lstep, ...} over an
    n-window of w columns: out free dims [g, w] with psum column
    t = t0 + lstep*k + n (overlapping within the op is fine -- PSUM
    accumulation is in-memory per element), and rhs free dims [g, w]
    reading y1[(t0 + delta - l0) + n, l0 + lstep*k].

    f32r mode: destinations must be 8-byte (even-element) aligned, so
    lstep=2 with delta = l0 % 2 routing odd l to a second accumulator
    representing out shifted left by one (tile col t == out[t + delta]).
    bf16 mode: no alignment restriction; lstep=1, delta=0, one accumulator.

    Windows split so each piece stays inside one PSUM bank (with a per-l
    fallback for boundary slivers); stop=True marks the last matmul
    touching each (delta, bank)."""
    if bf16mode:
        gmax, lstep, wwin, deltas = 4, 1, 128, (0,)
    else:
        gmax, lstep, wwin, deltas = 1, 2, 256, (0, 1)
    pieces = []
    for i in range(N // chunk):
        n0 = i * chunk
        for delta in deltas:
            lvals = list(range(delta, L)) if bf16mode else list(range(delta, L, 2))
            for gi in range(0, len(lvals), gmax):
                group = lvals[gi : gi + gmax]
                l0, g = group[0], len(group)
                for n_a in range(n0, n0 + chunk, wwin):
                    t0 = n_a + l0 - delta
                    remaining = min(wwin, n0 + chunk - n_a)
                    while remaining > 0:
                        bank_end = (t0 // BANK + 1) * BANK
                        w = min(remaining, bank_end - t0 - lstep * (g - 1))
                        if w < 1:
                            # group span straddles the bank boundary: emit
                            # the rest of this n-window per-l (small FD)
                            for k in range(g):
                                aa, rem2 = t0 + lstep * k, remaining
                                while rem2 > 0:
                                    be = (aa // BANK + 1) * BANK
                                    w2 = min(rem2, be - aa)
                                    pieces.append(
                                        [i, l0 + lstep * k, 1, lstep, delta,
                                         aa, w2, False]
                                    )
                                    aa += w2
                                    rem2 -= w2
                            break
                        pieces.append([i, l0, g, lstep, delta, t0, w, False])
                        t0 += w
                        remaining -= w
    last_by_bank = {}
    for idx, (_, _, _, _, delta, t0, _, _) in enumerate(pieces):
        last_by_bank[(delta, t0 // BANK)] = idx
    for idx in last_by_bank.values():
        pieces[idx][7] = True
    by_chunk = {}
    for i, l0, g, lstep, delta, t0, w, stop in pieces:
        by_chunk.setdefault(i, []).append((l0, g, lstep, delta, t0, w, stop))
    return by_chunk


def _build_nc(loop_iters=None, variant="full"):
    """Build the per-core Bass program. loop_iters wraps the body in an
    on-device For_i repeating the computation (for benchmarking); None
    runs it once. variant: "full", or "+"-joined flags out of
    {dma, tinydma, mul, pe} with optional "@<chunk>" suffix."""
    key = (loop_iters, variant)
    if key in _cached_nc:
        return _cached_nc[key]

    import concourse.bass as bass
    import concourse.mybir as mybir
    from concourse import bacc
    from concourse.tile import TileContext

    f32 = mybir.dt.float32
    f32r = mybir.dt.float32r
    nc = bacc.Bacc("TRN2")

    xin = nc.dram_tensor("x", (R, N * L), f32r, kind="ExternalInput")
    cain = nc.dram_tensor("ca", (R, N), f32, kind="ExternalInput")
    identin = nc.dram_tensor("ident", (R, R), f32r, kind="ExternalInput")
    outd = nc.dram_tensor("out", (R, ONC), f32, kind="ExternalOutput")

    vspec = variant
    chunk = CHUNK
    if "@" in vspec:
        vspec, csz = vspec.split("@")
        chunk = int(csz)
    if vspec == "full":
        flags = {"dma", "mul", "pe"}
    elif vspec == "bf16full":
        flags = {"dma", "mul", "pe", "bf16"}
    else:
        flags = set(vspec.split("+"))
    nchunks = N // chunk
    by_chunk = _shear_pieces(chunk, "bf16" in flags)
    xbufs = max(1, min(4, (150 * 1024) // (chunk * L * 4)))

    with TileContext(nc) as tc:
        with (
            tc.tile_pool(name="xp", bufs=xbufs) as xp,
            tc.tile_pool(name="cp", bufs=1) as cp,
            tc.tile_pool(name="accp", bufs=1) as accp,
            tc.tile_pool(name="pp", bufs=1, space="PSUM") as pp,
        ):
            ca_t = cp.tile([R, N], f32)
            nc.sync.dma_start(out=ca_t[:], in_=cain[:])
            id_t = cp.tile([R, R], f32r, tag="ident")
            nc.sync.dma_start(out=id_t[:], in_=identin[:])
            zw = cp.tile([R, R], f32, tag="zw")
            nc.gpsimd.memset(zw[:], 0.0)

            acc = accp.tile([R, ONC], f32)
            pacc_e = pp.tile([R, ONC], f32, tag="pe")
            pacc_o = pp.tile([R, ONC], f32, tag="po")
            paccs = (pacc_e, pacc_o)

            # "touch" ca on the vector engine so the first chunk's multiply
            # needs only one sync wait (TensorTensor has a single wait slot;
            # Bacc would otherwise spill onto an EventSemaphore nop)
            scr1 = cp.tile([R, 1], f32, tag="scr1")
            nc.vector.tensor_copy(scr1[:], ca_t[:, 0:1])

            bf16 = mybir.dt.bfloat16
            use_bf16 = "bf16" in flags
            if use_bf16:
                # bf16 identity for the shear matmuls (converted on-chip)
                idb = cp.tile([R, R], bf16, tag="idb")
                nc.scalar.copy(idb[:], id_t[:].bitcast(f32))

            def body():
                # Reset PSUM has_written bits and zero the accumulators: one
                # start=True zero-weight (plain fp32) matmul per bank.
                for pacc in (paccs if not use_bf16 else paccs[:1]):
                    for a in range(0, ONC, BANK):
                        b = min(a + BANK, ONC)
                        nc.tensor.matmul(
                            pacc[:, a:b],
                            zw[:],
                            ca_t[:, 0 : b - a],
                            start=True, stop=False, skip_group_check=True,
                        )
                for i in range(nchunks):
                    n0 = i * chunk
                    xt = xp.tile([R, chunk * L], f32r, tag="xchunk")
                    if "dma" in flags:
                        nc.sync.dma_start(
                            out=xt[:], in_=xin[:, n0 * L : (n0 + chunk) * L]
                        )
                    elif "tinydma" in flags:
                        nc.sync.dma_start(out=xt[:, 0:L], in_=xin[:, 0:L])

                    xv = xt[:]
                    yv = xv
                    if "mul" in flags:
                        x3 = xv.rearrange("p (n l) -> p n l", l=L)
                        cab = (
                            ca_t[:, n0 : n0 + chunk]
                            .unsqueeze(2)
                            .broadcast_to([R, chunk, L])
                        )
                        if use_bf16:
                            # y1 lands in a separate bf16 tile (fp32 reads,
                            # bf16 write is the fast conversion path)
                            y2 = xp.tile([R, chunk * L], bf16, tag="ychunk")
                            yv = y2[:]
                            y3 = yv.rearrange("p (n l) -> p n l", l=L)
                            nc.vector.tensor_tensor(
                                y3, x3.bitcast(f32), cab, mybir.AluOpType.mult
                            )
                        else:
                            # in-place; reads as plain f32, output carries
                            # the f32r rounding the PE consumers expect
                            nc.vector.tensor_tensor(
                                x3, x3.bitcast(f32), cab, mybir.AluOpType.mult
                            )

                    if "addpair" in flags:
                        # timing probe: per-l adds reading 8B-aligned PAIRS
                        # (emulates a pair-blocked y1 layout; numerics wrong
                        # on n-major data - use with tinydma only)
                        part = [int(xv.ap[0][0]), int(xv.ap[0][1])]
                        av0 = acc[:, 0:chunk]
                        pacc0 = [int(av0.ap[0][0]), int(av0.ap[0][1])]
                        for l in range(L):
                            src = bass.AP(
                                xv.tensor, xv.offset + 2 * l,
                                [part, [62, chunk // 2], [1, 2]],
                            )
                            dst = bass.AP(
                                av0.tensor, av0.offset + l,
                                [pacc0, [2, chunk // 2], [1, 2]],
                            )
                            nc.vector.tensor_tensor(
                                dst, src, dst, mybir.AluOpType.add
                            )
                    if "addl" in flags:
                        # timing probe: per-l adds with plain strided reads
                        part = [int(xv.ap[0][0]), int(xv.ap[0][1])]
                        av0 = acc[:, 0:chunk]
                        pacc0 = [int(av0.ap[0][0]), int(av0.ap[0][1])]
                        for l in range(L):
                            src = bass.AP(
                                xv.tensor, xv.offset + l, [part, [L, chunk]]
                            )
                            dst = bass.AP(
                                av0.tensor, av0.offset + l, [pacc0, [1, chunk]]
                            )
                            nc.vector.tensor_tensor(
                                dst, src, dst, mybir.AluOpType.add
                            )
                    if "pe" in flags:
                        part = [int(yv.ap[0][0]), int(yv.ap[0][1])]
                        wts = idb[:] if use_bf16 else id_t[:]
                        for l0, g, lstep, delta, t0, w, stop in by_chunk[i]:
                            # out col t = t0 + lstep*k + n (overlapping is
                            # fine; PSUM accumulation is in-memory); rhs
                            # elem (k,n) = y1[(t0+delta-l0)+n, l0+lstep*k]
                            rhs = bass.AP(
                                yv.tensor,
                                yv.offset + (t0 + delta - l0 - n0) * L + l0,
                                [part, [lstep, g], [L, w]],
                            )
                            pv = paccs[delta][:, t0 : t0 + lstep * (g - 1) + w]
                            pp0 = [int(pv.ap[0][0]), int(pv.ap[0][1])]
                            dst = bass.AP(
                                pv.tensor, pv.offset, [pp0, [lstep, g], [1, w]]
                            )
                            nc.tensor.matmul(
                                dst, wts, rhs,
                                start=False, stop=stop, skip_group_check=True,
                            )

                # evacuate PSUM -> SBUF (scalar engine); in f32r mode fold
                # in the odd-parity accumulator (shifted one column), DMA out
                nc.scalar.copy(acc[:], pacc_e[:])
                if not use_bf16:
                    nc.vector.tensor_tensor(
                        acc[:, 1:ONC], acc[:, 1:ONC], pacc_o[:, 0 : ONC - 1],
                        mybir.AluOpType.add,
                    )
                nc.sync.dma_start(out=outd[:], in_=acc[:])

            if loop_iters is None:
                body()
            else:
                with tc.For_i(0, loop_iters, 1):
                    body()

    nc.finalize()
    _cached_nc[key] = nc
    return nc


_IDENT = None


def _run(x_slab, ca_slab, loop_iters=None, variant="full", **run_kwargs):
    """x_slab (M, N*L) f32, ca_slab (M, N) f32 -> (M, ONC) f32."""
    from concourse.bass_utils import run_bass_kernel_spmd

    global _IDENT
    if _IDENT is None:
        _IDENT = np.eye(R, dtype=np.float32)

    nc = _build_nc(loop_iters, variant)
    in_maps = []
    for c in range(NCORES):
        in_maps.append(
            {
                "x": np.ascontiguousarray(x_slab[c * R : (c + 1) * R]),
                "ca": np.ascontiguousarray(ca_slab[c * R : (c + 1) * R]),
                "ident": _IDENT,
            }
        )
    res = run_bass_kernel_spmd(nc, in_maps, core_ids=list(range(NCORES)), **run_kwargs)
    out = np.concatenate(
        [np.asarray(res.results[c]["out"]) for c in range(NCORES)], axis=0
    )
    return out, res


def kernel(x, ca):
    x = np.ascontiguousarray(np.asarray(x, dtype=np.float32).reshape(M, N * L))
    ca = np.ascontiguousarray(np.asarray(ca, dtype=np.float32).reshape(M, N))
    out, _ = _run(x, ca)
    return out.reshape(1, M, ONC, 1)



# revision 16
# speedup vs baseline: 2.4298x; 2.4298x over previous
"""CASSI shear kernel for Trainium2 (Bass/Tile), 8-core SPMD.

Computes, for full inputs x (1, 1024, 1024, 31) and ca (1, 1024, 1024, 1):
    y1[m, n, l] = x[m, n, l] * ca[m, n]
    out[m, j]   = sum_{n+l=j} y1[m, n, l]       (j in [0, 1054))
returning (1, 1024, 1054, 1) float32.

Sharding: rows m across 8 cores (128 rows/core = one full SBUF partition
block). Per core, free dim holds the (n, l) plane contiguously (n-major,
matching HBM layout so DMA loads are fully contiguous per partition).

The kernel is HBM-bound: 16.25 MB of x per core per pass = ~45 us at the
358 GB/s per-core HBM limit. Everything else is sized to hide under that:

  - DMA (SWDGE): x is loaded in 8 chunks of 128 n-columns, cast f32->bf16
    in the DMA datapath (halves SBUF footprint, enables 2x DVE + full-rate
    PE). All 8 chunk tiles stay resident so DMA never stalls on compute.
  - Vector engine: y1 = x * cab in place, bf16 2x_1P mode (~2.2 us/chunk).
    cab is ca broadcast 31x along l, materialized once in the preamble by
    the (otherwise idle) scalar engine, per chunk slice so chunk 0 never
    waits on the whole build.
  - Tensor engine: the 31-way shear scatter-add as bf16 identity-weight
    matmuls accumulating into PSUM. One matmul covers ALL 31 l values over
    a 16-wide n window: out free dims [w=16, g=31] with psum column
    t = t0 + n + k (overlapping within the op is fine -- PSUM accumulation
    is in-memory per element), rhs free dims [w, g] = y1[t0-l0+n, l0+k],
    which is one FLAT CONTIGUOUS bf16 stream. Dim order matters: putting
    the stride-31 n dim innermost instead ran the PE ~4x slower (strided
    rhs fetch). Windows split at PSUM bank boundaries (per-l fallback at
    the two crossings). PSUM has_written is reset by marking the first
    matmul touching each bank start=True (clears the whole bank) -- no
    zero-weight reset matmuls needed.
  - Scalar engine: evacuates each PSUM bank to SBUF as soon as its last
    contribution lands (bank 0 halfway through, banks 1-2 at the end), so
    only the last chunk's compute + one small copy + store sit after the
    final DMA.

The benchmark loop (loop_iters=N) wraps the body in For_i with a PE
branch-prefetch hint (the body exceeds one IRAM block). Variants:
"full@uN" unrolls N bodies per back-edge (u4 was slower unprofiled --
IRAM thrash), "@s" staggered-reset stages (much slower -- stage barriers
break chunk pipelining), "@gN" regroups the shear l-dim.
"""

import sys

import numpy as np

if "/opt/trn_rl_repo" not in sys.path:
    sys.path.insert(0, "/opt/trn_rl_repo")

M, N, L = 1024, 1024, 31
ONC = N + L - 1  # 1054
NCORES = 8
R = M // NCORES  # 128 rows per core
CHUNK = 128
BANK = 512  # PSUM bank size in fp32 elements

_cached_nc = {}


def _shear_pieces(chunk, gmax=31):
    """All shear matmuls as {chunk_idx: [(l0, g, t0, w, start, stop)]}.

    Each matmul handles a group of g l-values {l0..l0+g-1} over the
    chunk's n-window: out free dims [g, w] with psum column t = t0 + k + n
    (overlapping within the op is fine -- PSUM accumulation is in-memory
    per element), rhs free dims [g, w] reading y1[(t0 - l0) + n, l0 + k].

    Windows split so each piece stays inside one PSUM bank. start=True
    marks the first matmul touching each bank (clears has_written for the
    whole bank -> accumulator resets with zero extra instructions);
    stop=True marks the last, gating that bank's evacuation.
    """
    pieces = []
    for i in range(N // chunk):
        n0 = i * chunk
        for l0 in range(0, L, gmax):
            g = min(gmax, L - l0)
            wmax = BANK // gmax  # keep out free size within one bank
            t0 = n0 + l0
            remaining = chunk
            while remaining > 0:
                bank_end = (t0 // BANK + 1) * BANK
                w = min(remaining, wmax, bank_end - t0 - (g - 1))
                if w < 1:
                    # group span straddles the bank boundary: emit the
                    # rest of this window per-l (small free dims)
                    for k in range(g):
                        aa, rem2 = t0 + k, remaining
                        while rem2 > 0:
                            be = (aa // BANK + 1) * BANK
                            w2 = min(rem2, be - aa)
                            pieces.append([i, l0 + k, 1, aa, w2, False, False])
                            aa += w2
                            rem2 -= w2
                    break
                pieces.append([i, l0, g, t0, w, False, False])
                t0 += w
                remaining -= w
    first_by_bank, last_by_bank = {}, {}
    for idx, (_, _, g, t0, w, _, _) in enumerate(pieces):
        # a piece touches banks floor(t0/BANK) .. floor((t0+g-1+w-1)/BANK);
        # by construction it stays in one bank
        b = t0 // BANK
        first_by_bank.setdefault(b, idx)
        last_by_bank[b] = idx
    for idx in first_by_bank.values():
        pieces[idx][5] = True
    for idx in last_by_bank.values():
        pieces[idx][6] = True
    by_chunk = {}
    for i, l0, g, t0, w, start, stop in pieces:
        by_chunk.setdefault(i, []).append((l0, g, t0, w, start, stop))
    # which banks see their final write in chunk i (drives evacuation)
    done_banks = {}
    for b, idx in last_by_bank.items():
        done_banks.setdefault(pieces[idx][0], []).append(b)
    return by_chunk, done_banks


def _build_nc(loop_iters=None, variant="full"):
    """Build the per-core Bass program. loop_iters wraps the body in an
    on-device For_i repeating the computation (for benchmarking); None
    runs it once. variant: "full", or "+"-joined flags out of
    {dma, mul, pe} with optional "@u<unroll>" suffix."""
    key = (loop_iters, variant)
    if key in _cached_nc:
        return _cached_nc[key]

    import concourse.bass as bass
    import concourse.mybir as mybir
    from concourse import bacc
    from concourse.tile import TileContext

    f32 = mybir.dt.float32
    bf16 = mybir.dt.bfloat16
    nc = bacc.Bacc("TRN2")

    xin = nc.dram_tensor("x", (R, N * L), f32, kind="ExternalInput")
    cain = nc.dram_tensor("ca", (R, N), f32, kind="ExternalInput")
    identin = nc.dram_tensor("ident", (R, R), f32, kind="ExternalInput")
    outd = nc.dram_tensor("out", (R, ONC), f32, kind="ExternalOutput")

    toks = variant.split("@")
    vspec = toks[0]
    unroll, gmax, staggered, hint = 1, 31, False, False
    for t in toks[1:]:
        if t == "s":
            staggered = True
        elif t == "h":
            hint = True
        elif t.startswith("u"):
            unroll = int(t[1:])
        elif t.startswith("g"):
            gmax = int(t[1:])
    if vspec == "full":
        flags = {"dma", "mul", "pe"}
    else:
        flags = set(vspec.split("+"))
    chunk = CHUNK
    nchunks = N // chunk
    by_chunk, done_banks = _shear_pieces(chunk, gmax)

    with TileContext(nc) as tc:
        with (
            tc.tile_pool(name="xp", bufs=nchunks) as xp,
            tc.tile_pool(name="cp", bufs=1) as cp,
            tc.tile_pool(name="accp", bufs=2) as accp,
            tc.tile_pool(name="pp", bufs=2, space="PSUM") as pp,
        ):
            ca_t = cp.tile([R, N], f32)
            nc.sync.dma_start(out=ca_t[:], in_=cain[:])
            # bf16 identity for the shear matmuls (cast during DMA)
            idb = cp.tile([R, R], bf16, tag="idb")
            nc.gpsimd.dma_start(out=idb[:], in_=identin[:])

            # cab[m, n*L + l] = ca[m, n] as bf16: built once, per chunk
            # slice, on the scalar engine (idle during the main loop)
            cab = cp.tile([R, N * L], bf16, tag="cab")
            cab3 = cab[:].rearrange("p (n l) -> p n l", l=L)
            for i in range(nchunks):
                n0 = i * chunk
                src = (
                    ca_t[:, n0 : n0 + chunk]
                    .unsqueeze(2)
                    .broadcast_to([R, chunk, L])
                )
                nc.scalar.copy(cab3[:, n0 : n0 + chunk], src)

            def body(marks=()):
                pacc = pp.tile([R, ONC], f32, tag="pacc")
                for i in range(nchunks):
                    if i in marks:
                        tc.stage_boundary()
                    n0 = i * chunk
                    xt = xp.tile([R, chunk * L], bf16, tag="xchunk")
                    if "dma" in flags:
                        # f32 -> bf16 cast in the DMA datapath (SWDGE)
                        nc.gpsimd.dma_start(
                            out=xt[:], in_=xin[:, n0 * L : (n0 + chunk) * L]
                        )
                    yv = xt[:]
                    if "mul" in flags:
                        # in-place broadcast multiply, bf16 2x_1P (both
                        # operands contiguous step-1 bf16)
                        nc.vector.tensor_tensor(
                            yv,
                            yv,
                            cab[:, n0 * L : (n0 + chunk) * L],
                            mybir.AluOpType.mult,
                        )
                    if "pe" in flags:
                        part = [int(yv.ap[0][0]), int(yv.ap[0][1])]
                        for l0, g, t0, w, start, stop in by_chunk[i]:
                            # out col t = t0 + k + n (overlap inside the
                            # op is fine; PSUM accumulation is in-memory);
                            # rhs elem (n,k) = y1[(t0-l0-n0)+n, l0+k].
                            # Dim order: n outer, l-group inner -- the
                            # innermost run is contiguous in SBUF (the PE
                            # rhs fetch rate collapses on strided inner
                            # reads; at g=31 the whole stream is flat)
                            rhs = bass.AP(
                                yv.tensor,
                                yv.offset + (t0 - l0 - n0) * L + l0,
                                [part, [L, w], [1, g]],
                            )
                            pv = pacc[:, t0 : t0 + (g - 1) + w]
                            pp0 = [int(pv.ap[0][0]), int(pv.ap[0][1])]
                            dst = bass.AP(
                                pv.tensor, pv.offset, [pp0, [1, w], [1, g]]
                            )
                            nc.tensor.matmul(
                                dst,
                                idb[:],
                                rhs,
                                start=start,
                                stop=stop,
                                skip_group_check=True,
                            )
                        # evacuate any PSUM banks whose last contribution
                        # just landed (adjacent banks coalesced); store
                        # them right away
                        bs = sorted(done_banks.get(i, []))
                        while bs:
                            b0 = b1 = bs.pop(0)
                            while bs and bs[0] == b1 + 1:
                                b1 = bs.pop(0)
                            a0 = b0 * BANK
                            a1 = min((b1 + 1) * BANK, ONC)
                            at = accp.tile([R, a1 - a0], f32, tag=f"acc{b0}")
                            nc.scalar.copy(at[:], pacc[:, a0:a1])
                            nc.sync.dma_start(
                                out=outd[:, a0:a1], in_=at[:]
                            )

            if loop_iters is None:
                body()
            else:
                u = max(u for u in (unroll, 1) if loop_iters % u == 0)
                hints = (mybir.EngineType.PE,) if hint else ()
                stag = staggered and (u * nchunks) % 4 == 0
                # staggered_reset needs exactly 4 stages per loop body;
                # spread them evenly over the unrolled copies' chunks
                span = (u * nchunks) // 4
                with tc.For_i(
                    0, loop_iters // u, 1,
                    hint_engines=hints, staggered_reset=stag,
                ):
                    for j in range(u):
                        if stag:
                            marks = {
                                (s * span) - j * nchunks
                                for s in range(1, 4)
                                if 0 <= (s * span) - j * nchunks < nchunks
                            }
                        else:
                            marks = ()
                        body(marks)

    nc.finalize()
    _cached_nc[key] = nc
    return nc


_IDENT = None


def _run(x_slab, ca_slab, loop_iters=None, variant="full", **run_kwargs):
    """x_slab (M, N*L) f32, ca_slab (M, N) f32 -> (M, ONC) f32."""
    from concourse.bass_utils import run_bass_kernel_spmd

    global _IDENT
    if _IDENT is None:
        _IDENT = np.eye(R, dtype=np.float32)

    nc = _build_nc(loop_iters, variant)
    in_maps = []
    for c in range(NCORES):
        in_maps.append(
            {
                "x": np.ascontiguousarray(x_slab[c * R : (c + 1) * R]),
                "ca": np.ascontiguousarray(ca_slab[c * R : (c + 1) * R]),
                "ident": _IDENT,
            }
        )
    res = run_bass_kernel_spmd(nc, in_maps, core_ids=list(range(NCORES)), **run_kwargs)
    out = np.concatenate(
        [np.asarray(res.results[c]["out"]) for c in range(NCORES)], axis=0
    )
    return out, res


def kernel(x, ca):
    x = np.ascontiguousarray(np.asarray(x, dtype=np.float32).reshape(M, N * L))
    ca = np.ascontiguousarray(np.asarray(ca, dtype=np.float32).reshape(M, N))
    out, _ = _run(x, ca)
    return out.reshape(1, M, ONC, 1)


# revision 22
# speedup vs baseline: 2.4984x; 1.0282x over previous
"""CASSI shear kernel for Trainium2 (Bass/Tile), 8-core SPMD.

Computes, for full inputs x (1, 1024, 1024, 31) and ca (1, 1024, 1024, 1):
    y1[m, n, l] = x[m, n, l] * ca[m, n]
    out[m, j]   = sum_{n+l=j} y1[m, n, l]       (j in [0, 1054))
returning (1, 1024, 1054, 1) float32.

Sharding: rows m across 8 cores (128 rows/core = one full SBUF partition
block). Per core, free dim holds the (n, l) plane contiguously (n-major,
matching HBM layout so DMA loads are fully contiguous per partition).

The kernel is HBM-bound: 16.25 MB of x per core per pass = ~45 us at the
358 GB/s per-core HBM limit. Everything else is sized to hide under that:

  - DMA (SWDGE): x is loaded in 8 chunks of 128 n-columns, cast f32->bf16
    in the DMA datapath (halves SBUF footprint, enables 2x DVE + full-rate
    PE). All 8 chunk tiles stay resident so DMA never stalls on compute.
  - Vector engine: y1 = x * cab in place, bf16 2x_1P mode (~2.2 us/chunk).
    cab is ca broadcast 31x along l, materialized once in the preamble by
    the (otherwise idle) scalar engine, per chunk slice so chunk 0 never
    waits on the whole build.
  - Tensor engine: the 31-way shear scatter-add as bf16 identity-weight
    matmuls accumulating into PSUM. One matmul covers ALL 31 l values over
    a 16-wide n window: out free dims [w=16, g=31] with psum column
    t = t0 + n + k (overlapping within the op is fine -- PSUM accumulation
    is in-memory per element), rhs free dims [w, g] = y1[t0-l0+n, l0+k],
    which is one FLAT CONTIGUOUS bf16 stream. Dim order matters: putting
    the stride-31 n dim innermost instead ran the PE ~4x slower (strided
    rhs fetch). Windows split at PSUM bank boundaries (per-l fallback at
    the two crossings). PSUM has_written is reset by marking the first
    matmul touching each bank start=True (clears the whole bank) -- no
    zero-weight reset matmuls needed.
  - Scalar engine: evacuates each PSUM bank to SBUF as soon as its last
    contribution lands (bank 0 halfway through, banks 1-2 at the end), so
    only the last chunk's compute + one small copy + store sit after the
    final DMA.

The benchmark loop (loop_iters=N) wraps the body in For_i with a PE
branch-prefetch hint (the body exceeds one IRAM block). Variants:
"full@uN" unrolls N bodies per back-edge (u4 was slower unprofiled --
IRAM thrash), "@s" staggered-reset stages (much slower -- stage barriers
break chunk pipelining), "@gN" regroups the shear l-dim.
"""

import sys

import numpy as np

if "/opt/trn_rl_repo" not in sys.path:
    sys.path.insert(0, "/opt/trn_rl_repo")

M, N, L = 1024, 1024, 31
ONC = N + L - 1  # 1054
NCORES = 8
R = M // NCORES  # 128 rows per core
CHUNK = 128
BANK = 512  # PSUM bank size in fp32 elements

_cached_nc = {}


def _shear_pieces(chunk, gmax=31):
    """All shear matmuls as {chunk_idx: [(l0, g, t0, w, start, stop)]}.

    Each matmul handles a group of g l-values {l0..l0+g-1} over the
    chunk's n-window: out free dims [g, w] with psum column t = t0 + k + n
    (overlapping within the op is fine -- PSUM accumulation is in-memory
    per element), rhs free dims [g, w] reading y1[(t0 - l0) + n, l0 + k].

    Windows split so each piece stays inside one PSUM bank. start=True
    marks the first matmul touching each bank (clears has_written for the
    whole bank -> accumulator resets with zero extra instructions);
    stop=True marks the last, gating that bank's evacuation.
    """
    if isinstance(chunk, int):
        chunks = [(i * chunk, chunk) for i in range(N // chunk)]
    else:
        chunks = chunk
    pieces = []
    for i, (n0, cw) in enumerate(chunks):
        for l0 in range(0, L, gmax):
            g = min(gmax, L - l0)
            wmax = BANK // gmax  # keep out free size within one bank
            t0 = n0 + l0
            remaining = cw
            while remaining > 0:
                bank_end = (t0 // BANK + 1) * BANK
                w = min(remaining, wmax, bank_end - t0 - (g - 1))
                if w < 1:
                    # group span straddles the bank boundary: emit the
                    # rest of this window per-l (small free dims)
                    for k in range(g):
                        aa, rem2 = t0 + k, remaining
                        while rem2 > 0:
                            be = (aa // BANK + 1) * BANK
                            w2 = min(rem2, be - aa)
                            pieces.append([i, l0 + k, 1, aa, w2, False, False])
                            aa += w2
                            rem2 -= w2
                    break
                pieces.append([i, l0, g, t0, w, False, False])
                t0 += w
                remaining -= w
    first_by_bank, last_by_bank = {}, {}
    for idx, (_, _, g, t0, w, _, _) in enumerate(pieces):
        # a piece touches banks floor(t0/BANK) .. floor((t0+g-1+w-1)/BANK);
        # by construction it stays in one bank
        b = t0 // BANK
        first_by_bank.setdefault(b, idx)
        last_by_bank[b] = idx
    for idx in first_by_bank.values():
        pieces[idx][5] = True
    for idx in last_by_bank.values():
        pieces[idx][6] = True
    by_chunk = {}
    for i, l0, g, t0, w, start, stop in pieces:
        by_chunk.setdefault(i, []).append((l0, g, t0, w, start, stop))
    # which banks see their final write in chunk i (drives evacuation)
    done_banks = {}
    for b, idx in last_by_bank.items():
        done_banks.setdefault(pieces[idx][0], []).append(b)
    return by_chunk, done_banks


def _build_nc(loop_iters=None, variant="full"):
    """Build the per-core Bass program. loop_iters wraps the body in an
    on-device For_i repeating the computation (for benchmarking); None
    runs it once. variant: "full", or "+"-joined flags out of
    {dma, mul, pe} with optional "@u<unroll>" suffix."""
    key = (loop_iters, variant)
    if key in _cached_nc:
        return _cached_nc[key]

    import concourse.bass as bass
    import concourse.mybir as mybir
    from concourse import bacc
    from concourse.tile import TileContext

    f32 = mybir.dt.float32
    bf16 = mybir.dt.bfloat16
    nc = bacc.Bacc("TRN2")

    xin = nc.dram_tensor("x", (R, N * L), f32, kind="ExternalInput")
    cain = nc.dram_tensor("ca", (R, N), f32, kind="ExternalInput")
    identin = nc.dram_tensor("ident", (R, R), f32, kind="ExternalInput")
    outd = nc.dram_tensor("out", (R, ONC), f32, kind="ExternalOutput")

    toks = variant.split("@")
    vspec = toks[0]
    unroll, gmax, staggered, hint, tapered = 1, 31, False, False, False
    for t in toks[1:]:
        if t == "s":
            staggered = True
        elif t == "h":
            hint = True
        elif t == "t":
            tapered = True
        elif t.startswith("u"):
            unroll = int(t[1:])
        elif t.startswith("g"):
            gmax = int(t[1:])
    if vspec == "full":
        flags = {"dma", "mul", "pe"}
    else:
        flags = set(vspec.split("+"))
    if tapered:
        # split the last 128-col chunk in two: halves the serial tail
        # (mul + shear of the final chunk) behind the last DMA
        chunks = [(i * CHUNK, CHUNK) for i in range(N // CHUNK - 1)]
        h = CHUNK // 2
        chunks += [(N - CHUNK, h), (N - h, h)]
    else:
        chunks = [(i * CHUNK, CHUNK) for i in range(N // CHUNK)]
    nchunks = len(chunks)
    by_chunk, done_banks = _shear_pieces(chunks, gmax)

    with TileContext(nc) as tc:
        with (
            tc.tile_pool(name="xp", bufs=nchunks) as xp,
            tc.tile_pool(name="cp", bufs=1) as cp,
            tc.tile_pool(name="accp", bufs=2) as accp,
            tc.tile_pool(name="pp", bufs=2, space="PSUM") as pp,
        ):
            ca_t = cp.tile([R, N], f32)
            nc.sync.dma_start(out=ca_t[:], in_=cain[:])
            # bf16 identity for the shear matmuls (cast during DMA)
            idb = cp.tile([R, R], bf16, tag="idb")
            nc.gpsimd.dma_start(out=idb[:], in_=identin[:])

            # cab[m, n*L + l] = ca[m, n] as bf16: built once, per chunk
            # slice, on the scalar engine (idle during the main loop)
            cab = cp.tile([R, N * L], bf16, tag="cab")
            cab3 = cab[:].rearrange("p (n l) -> p n l", l=L)
            for n0, cw in chunks:
                src = (
                    ca_t[:, n0 : n0 + cw]
                    .unsqueeze(2)
                    .broadcast_to([R, cw, L])
                )
                nc.scalar.copy(cab3[:, n0 : n0 + cw], src)

            def body(marks=()):
                pacc = pp.tile([R, ONC], f32, tag="pacc")
                for i, (n0, cw) in enumerate(chunks):
                    if i in marks:
                        tc.stage_boundary()
                    xt = xp.tile([R, cw * L], bf16, tag="xchunk")
                    if "dma" in flags:
                        # f32 -> bf16 cast in the DMA datapath (SWDGE)
                        nc.gpsimd.dma_start(
                            out=xt[:], in_=xin[:, n0 * L : (n0 + cw) * L]
                        )
                    yv = xt[:]
                    if "mul" in flags:
                        # in-place broadcast multiply, bf16 2x_1P (both
                        # operands contiguous step-1 bf16)
                        nc.vector.tensor_tensor(
                            yv,
                            yv,
                            cab[:, n0 * L : (n0 + cw) * L],
                            mybir.AluOpType.mult,
                        )
                    if "pe" in flags:
                        part = [int(yv.ap[0][0]), int(yv.ap[0][1])]
                        for l0, g, t0, w, start, stop in by_chunk[i]:
                            # out col t = t0 + k + n (overlap inside the
                            # op is fine; PSUM accumulation is in-memory);
                            # rhs elem (n,k) = y1[(t0-l0-n0)+n, l0+k].
                            # Dim order: n outer, l-group inner -- the
                            # innermost run is contiguous in SBUF (the PE
                            # rhs fetch rate collapses on strided inner
                            # reads; at g=31 the whole stream is flat)
                            rhs = bass.AP(
                                yv.tensor,
                                yv.offset + (t0 - l0 - n0) * L + l0,
                                [part, [L, w], [1, g]],
                            )
                            pv = pacc[:, t0 : t0 + (g - 1) + w]
                            pp0 = [int(pv.ap[0][0]), int(pv.ap[0][1])]
                            dst = bass.AP(
                                pv.tensor, pv.offset, [pp0, [1, w], [1, g]]
                            )
                            nc.tensor.matmul(
                                dst,
                                idb[:],
                                rhs,
                                start=start,
                                stop=stop,
                                skip_group_check=True,
                            )
                        # evacuate any PSUM banks whose last contribution
                        # just landed (adjacent banks coalesced); store
                        # them right away
                        bs = sorted(done_banks.get(i, []))
                        while bs:
                            b0 = b1 = bs.pop(0)
                            while bs and bs[0] == b1 + 1:
                                b1 = bs.pop(0)
                            a0 = b0 * BANK
                            a1 = min((b1 + 1) * BANK, ONC)
                            at = accp.tile([R, a1 - a0], f32, tag=f"acc{b0}")
                            nc.scalar.copy(at[:], pacc[:, a0:a1])
                            nc.sync.dma_start(
                                out=outd[:, a0:a1], in_=at[:]
                            )

            if loop_iters is None:
                body()
            else:
                u = max(u for u in (unroll, 1) if loop_iters % u == 0)
                hints = (mybir.EngineType.PE,) if hint else ()
                stag = staggered and (u * nchunks) % 4 == 0
                # staggered_reset needs exactly 4 stages per loop body;
                # spread them evenly over the unrolled copies' chunks
                span = (u * nchunks) // 4
                with tc.For_i(
                    0, loop_iters // u, 1,
                    hint_engines=hints, staggered_reset=stag,
                ):
                    for j in range(u):
                        if stag:
                            marks = {
                                (s * span) - j * nchunks
                                for s in range(1, 4)
                                if 0 <= (s * span) - j * nchunks < nchunks
                            }
                        else:
                            marks = ()
                        body(marks)

    nc.finalize()
    _cached_nc[key] = nc
    return nc


_IDENT = None


def _run(x_slab, ca_slab, loop_iters=None, variant="full", **run_kwargs):
    """x_slab (M, N*L) f32, ca_slab (M, N) f32 -> (M, ONC) f32."""
    from concourse.bass_utils import run_bass_kernel_spmd

    global _IDENT
    if _IDENT is None:
        _IDENT = np.eye(R, dtype=np.float32)

    nc = _build_nc(loop_iters, variant)
    in_maps = []
    for c in range(NCORES):
        in_maps.append(
            {
                "x": np.ascontiguousarray(x_slab[c * R : (c + 1) * R]),
                "ca": np.ascontiguousarray(ca_slab[c * R : (c + 1) * R]),
                "ident": _IDENT,
            }
        )
    res = run_bass_kernel_spmd(nc, in_maps, core_ids=list(range(NCORES)), **run_kwargs)
    out = np.concatenate(
        [np.asarray(res.results[c]["out"]) for c in range(NCORES)], axis=0
    )
    return out, res


def kernel(x, ca):
    x = np.ascontiguousarray(np.asarray(x, dtype=np.float32).reshape(M, N * L))
    ca = np.ascontiguousarray(np.asarray(ca, dtype=np.float32).reshape(M, N))
    out, _ = _run(x, ca)
    return out.reshape(1, M, ONC, 1)


# revision 35
# speedup vs baseline: 2.9929x; 1.1979x over previous
"""CASSI shear kernel for Trainium2 (Bass/Tile), 8-core SPMD.

Computes, for full inputs x (1, 1024, 1024, 31) and ca (1, 1024, 1024, 1):
    y1[m, n, l] = x[m, n, l] * ca[m, n]
    out[m, j]   = sum_{n+l=j} y1[m, n, l]       (j in [0, 1054))
returning (1, 1024, 1054, 1) float32.

Sharding: rows m across 8 cores (128 rows/core = one full SBUF partition
block). Per core, free dim holds the (n, l) plane contiguously (n-major,
matching HBM layout so DMA loads are fully contiguous per partition).

The kernel is HBM-bound: 16.25 MB of x per core per pass = ~45 us at the
358 GB/s per-core HBM limit. Everything else is sized to hide under that:

  - DMA (SWDGE): x is loaded in 8 chunks of 128 n-columns, cast f32->bf16
    in the DMA datapath (halves SBUF footprint, enables 2x DVE + full-rate
    PE). All 8 chunk tiles stay resident so DMA never stalls on compute.
  - Vector engine: y1 = x * cab in place, bf16 2x_1P mode (~2.2 us/chunk).
    cab is ca broadcast 31x along l, materialized once in the preamble by
    the (otherwise idle) scalar engine, per chunk slice so chunk 0 never
    waits on the whole build.
  - Tensor engine: the 31-way shear scatter-add as bf16 identity-weight
    matmuls accumulating into PSUM. One matmul covers ALL 31 l values over
    a 16-wide n window: out free dims [w=16, g=31] with psum column
    t = t0 + n + k (overlapping within the op is fine -- PSUM accumulation
    is in-memory per element), rhs free dims [w, g] = y1[t0-l0+n, l0+k],
    which is one FLAT CONTIGUOUS bf16 stream. Dim order matters: putting
    the stride-31 n dim innermost instead ran the PE ~4x slower (strided
    rhs fetch). Windows split at PSUM bank boundaries (per-l fallback at
    the two crossings). PSUM has_written is reset by marking the first
    matmul touching each bank start=True (clears the whole bank) -- no
    zero-weight reset matmuls needed.
  - Scalar engine: evacuates each PSUM bank to SBUF as soon as its last
    contribution lands (bank 0 halfway through, banks 1-2 at the end), so
    only the last chunk's compute + one small copy + store sit after the
    final DMA.

The benchmark loop (loop_iters=N) wraps the body in For_i with a PE
branch-prefetch hint (the body exceeds one IRAM block). Variants:
"full@uN" unrolls N bodies per back-edge (u4 was slower unprofiled --
IRAM thrash), "@s" staggered-reset stages (much slower -- stage barriers
break chunk pipelining), "@gN" regroups the shear l-dim.
"""

import sys

import numpy as np

if "/opt/trn_rl_repo" not in sys.path:
    sys.path.insert(0, "/opt/trn_rl_repo")

M, N, L = 1024, 1024, 31
ONC = N + L - 1  # 1054
NCORES = 8
R = M // NCORES  # 128 rows per core
CHUNK = 128
BANK = 512  # PSUM bank size in fp32 elements

_cached_nc = {}


def _shear_pieces(chunk, gmax=31, carry=False):
    """All shear matmuls as {chunk_idx: [(l0, g, t0, w, start, stop)]}.

    Each matmul handles a group of g l-values {l0..l0+g-1} over the
    chunk's n-window: out free dims [g, w] with psum column t = t0 + k + n
    (overlapping within the op is fine -- PSUM accumulation is in-memory
    per element), rhs free dims [g, w] reading y1[(t0 - l0) + n, l0 + k].

    Windows split so each piece stays inside one PSUM bank. start=True
    marks the first matmul touching each bank (clears has_written for the
    whole bank -> accumulator resets with zero extra instructions);
    stop=True marks the last, gating that bank's evacuation.
    """
    if isinstance(chunk, int):
        chunks = [(i * chunk, chunk) for i in range(N // chunk)]
    else:
        chunks = chunk
    pieces = []
    for i, (n0, cw) in enumerate(chunks):
        for l0 in range(0, L, gmax):
            g = min(gmax, L - l0)
            wmax = BANK // gmax  # keep out free size within one bank
            t0 = n0 + l0
            remaining = cw
            while remaining > 0:
                bank_end = (t0 // BANK + 1) * BANK
                w = min(remaining, wmax, bank_end - t0 - (g - 1))
                if w < 1:
                    if carry:
                        # group span straddles the bank boundary: route
                        # the whole straddle rectangle (all g l's, the
                        # n's whose span crosses) into the carry bank as
                        # ONE matmul; merged into the output during
                        # evacuation. Carry col = CARRY_OFF[be] + (out
                        # col - (be - (g-1) - 1))... here simply
                        # cdst = carry base + (t0 - (be - 30)).
                        # out free size (wc*g) must stay <= 512 (fp32
                        # PSUM ISA limit), so split the straddle region
                        # into wmax-wide windows like the main pieces
                        wc = min(remaining, bank_end - t0, wmax)
                        cbase = 0 if bank_end == BANK else 64
                        cdst = cbase + (t0 - (bank_end - 30))
                        assert 0 <= cdst and cdst + wc - 1 + (g - 1) < cbase + 60
                        pieces.append(
                            [i, l0, g, t0, wc, False, False, cdst]
                        )
                        t0 += wc
                        remaining -= wc
                        continue
                    # no-carry fallback: emit the rest of this window
                    # per-l (small free dims)
                    for k in range(g):
                        aa, rem2 = t0 + k, remaining
                        while rem2 > 0:
                            be = (aa // BANK + 1) * BANK
                            w2 = min(rem2, be - aa)
                            pieces.append(
                                [i, l0 + k, 1, aa, w2, False, False, None]
                            )
                            aa += w2
                            rem2 -= w2
                    break
                pieces.append([i, l0, g, t0, w, False, False, None])
                t0 += w
                remaining -= w
    first_by_bank, last_by_bank = {}, {}
    for idx, (_, _, g, t0, w, _, _, cdst) in enumerate(pieces):
        # a piece touches banks floor(t0/BANK) .. floor((t0+g-1+w-1)/BANK);
        # by construction it stays in one bank ("carry" = the carry bank)
        b = "carry" if cdst is not None else t0 // BANK
        first_by_bank.setdefault(b, idx)
        last_by_bank[b] = idx
    for idx in first_by_bank.values():
        pieces[idx][5] = True
    for idx in last_by_bank.values():
        pieces[idx][6] = True
    by_chunk = {}
    for i, l0, g, t0, w, start, stop, cdst in pieces:
        by_chunk.setdefault(i, []).append((l0, g, t0, w, start, stop, cdst))
    # which banks see their final write in chunk i (drives evacuation)
    done_banks = {}
    for b, idx in last_by_bank.items():
        if b != "carry":
            done_banks.setdefault(pieces[idx][0], []).append(b)
    return by_chunk, done_banks


def _build_nc(loop_iters=None, variant="full"):
    """Build the per-core Bass program. loop_iters wraps the body in an
    on-device For_i repeating the computation (for benchmarking); None
    runs it once. variant: "full", or "+"-joined flags out of
    {dma, mul, pe} with optional "@u<unroll>" suffix."""
    key = (loop_iters, variant)
    if key in _cached_nc:
        return _cached_nc[key]

    import concourse.bass as bass
    import concourse.mybir as mybir
    from concourse import bacc
    from concourse.tile import TileContext

    f32 = mybir.dt.float32
    bf16 = mybir.dt.bfloat16
    nc = bacc.Bacc("TRN2")

    xin = nc.dram_tensor("x", (R, N * L), f32, kind="ExternalInput")
    cain = nc.dram_tensor("ca", (R, N), f32, kind="ExternalInput")
    identin = nc.dram_tensor("ident", (R, R), f32, kind="ExternalInput")
    outd = nc.dram_tensor("out", (R, ONC), f32, kind="ExternalOutput")

    toks = variant.split("@")
    vspec = toks[0]
    unroll, gmax, staggered, hint, tapered, use_carry = 1, 31, False, False, False, True
    for t in toks[1:]:
        if t == "s":
            staggered = True
        elif t == "h":
            hint = True
        elif t == "t":
            tapered = True
        elif t == "c":
            use_carry = True
        elif t == "nc":
            use_carry = False
        elif t.startswith("u"):
            unroll = int(t[1:])
        elif t.startswith("g"):
            gmax = int(t[1:])
    # the hardcoded carry-merge geometry only exists for the default tiling
    if tapered or gmax != 31:
        use_carry = False
    if vspec == "full":
        flags = {"dma", "mul", "pe"}
    else:
        flags = set(vspec.split("+"))
    if tapered:
        # split the last 128-col chunk in two: halves the serial tail
        # (mul + shear of the final chunk) behind the last DMA
        chunks = [(i * CHUNK, CHUNK) for i in range(N // CHUNK - 1)]
        h = CHUNK // 2
        chunks += [(N - CHUNK, h), (N - h, h)]
    else:
        chunks = [(i * CHUNK, CHUNK) for i in range(N // CHUNK)]
    nchunks = len(chunks)
    by_chunk, done_banks = _shear_pieces(chunks, gmax, carry=use_carry)
    if use_carry:
        # hardcoded merge geometry below assumes this piece layout
        assert not tapered and gmax == 31 and CHUNK == 128
        assert sorted(done_banks.get(3, [])) == [0]
        assert sorted(done_banks.get(7, [])) == [1]

    with TileContext(nc) as tc:
        with (
            tc.tile_pool(name="xp", bufs=nchunks) as xp,
            tc.tile_pool(name="cp", bufs=1) as cp,
            tc.tile_pool(name="accp", bufs=2) as accp,
            tc.tile_pool(name="pp", bufs=2, space="PSUM") as pp,
        ):
            ca_t = cp.tile([R, N], f32)
            nc.sync.dma_start(out=ca_t[:], in_=cain[:])
            # bf16 identity for the shear matmuls (cast during DMA)
            idb = cp.tile([R, R], bf16, tag="idb")
            nc.gpsimd.dma_start(out=idb[:], in_=identin[:])

            # cab[m, n*L + l] = ca[m, n] as bf16: built once, per chunk
            # slice, on the scalar engine (idle during the main loop)
            cab = cp.tile([R, N * L], bf16, tag="cab")
            cab3 = cab[:].rearrange("p (n l) -> p n l", l=L)
            for n0, cw in chunks:
                src = (
                    ca_t[:, n0 : n0 + cw]
                    .unsqueeze(2)
                    .broadcast_to([R, cw, L])
                )
                nc.scalar.copy(cab3[:, n0 : n0 + cw], src)

            def body(marks=()):
                pacc = pp.tile([R, ONC], f32, tag="pacc")
                pc = None
                if use_carry:
                    pc = pp.tile([R, 128], f32, tag="carry")
                for i, (n0, cw) in enumerate(chunks):
                    if i in marks:
                        tc.stage_boundary()
                    xt = xp.tile([R, cw * L], bf16, tag="xchunk")
                    if "dma" in flags:
                        # f32 -> bf16 cast in the DMA datapath (SWDGE)
                        nc.gpsimd.dma_start(
                            out=xt[:], in_=xin[:, n0 * L : (n0 + cw) * L]
                        )
                    yv = xt[:]
                    if "mul" in flags:
                        # in-place broadcast multiply, bf16 2x_1P (both
                        # operands contiguous step-1 bf16)
                        nc.vector.tensor_tensor(
                            yv,
                            yv,
                            cab[:, n0 * L : (n0 + cw) * L],
                            mybir.AluOpType.mult,
                        )
                    if "pe" in flags:
                        part = [int(yv.ap[0][0]), int(yv.ap[0][1])]
                        for l0, g, t0, w, start, stop, cdst in by_chunk[i]:
                            # out col t = t0 + k + n (overlap inside the
                            # op is fine; PSUM accumulation is in-memory);
                            # rhs elem (n,k) = y1[(t0-l0-n0)+n, l0+k].
                            # Dim order: n outer, l-group inner -- the
                            # innermost run is contiguous in SBUF (the PE
                            # rhs fetch rate collapses on strided inner
                            # reads; at g=31 the whole stream is flat)
                            rhs = bass.AP(
                                yv.tensor,
                                yv.offset + (t0 - l0 - n0) * L + l0,
                                [part, [L, w], [1, g]],
                            )
                            if cdst is None:
                                pv = pacc[:, t0 : t0 + (g - 1) + w]
                            else:
                                pv = pc[:, cdst : cdst + (g - 1) + w]
                            pp0 = [int(pv.ap[0][0]), int(pv.ap[0][1])]
                            dst = bass.AP(
                                pv.tensor, pv.offset, [pp0, [1, w], [1, g]]
                            )
                            nc.tensor.matmul(
                                dst,
                                idb[:],
                                rhs,
                                start=start,
                                stop=stop,
                                skip_group_check=True,
                            )
                        if use_carry:
                            # explicit evacuation with carry merges
                            # (geometry asserted above)
                            add = mybir.AluOpType.add
                            if i == 3:
                                at0 = accp.tile([R, BANK], f32, tag="acc0")
                                nc.scalar.copy(at0[:], pacc[:, 0:BANK])
                                nc.vector.tensor_tensor(
                                    at0[:, 482:512], at0[:, 482:512],
                                    pc[:, 0:30], add,
                                )
                                nc.sync.dma_start(
                                    out=outd[:, 0:BANK], in_=at0[:]
                                )
                            elif i == 7:
                                at1 = accp.tile([R, ONC - BANK], f32,
                                                tag="acc1")
                                nc.scalar.copy(
                                    at1[:, 0:512], pacc[:, 512:1024]
                                )
                                nc.vector.tensor_tensor(
                                    at1[:, 0:30], at1[:, 0:30],
                                    pc[:, 30:60], add,
                                )
                                nc.vector.tensor_tensor(
                                    at1[:, 482:512], at1[:, 482:512],
                                    pc[:, 64:94], add,
                                )
                                nc.vector.tensor_copy(
                                    at1[:, 512:542], pc[:, 94:124]
                                )
                                nc.sync.dma_start(
                                    out=outd[:, BANK:ONC], in_=at1[:]
                                )
                            continue
                        # evacuate any PSUM banks whose last contribution
                        # just landed (adjacent banks coalesced); store
                        # them right away
                        bs = sorted(done_banks.get(i, []))
                        while bs:
                            b0 = b1 = bs.pop(0)
                            while bs and bs[0] == b1 + 1:
                                b1 = bs.pop(0)
                            a0 = b0 * BANK
                            a1 = min((b1 + 1) * BANK, ONC)
                            at = accp.tile([R, a1 - a0], f32, tag=f"acc{b0}")
                            nc.scalar.copy(at[:], pacc[:, a0:a1])
                            nc.sync.dma_start(
                                out=outd[:, a0:a1], in_=at[:]
                            )

            if loop_iters is None:
                body()
            else:
                u = max(u for u in (unroll, 1) if loop_iters % u == 0)
                hints = (mybir.EngineType.PE,) if hint else ()
                stag = staggered and (u * nchunks) % 4 == 0
                # staggered_reset needs exactly 4 stages per loop body;
                # spread them evenly over the unrolled copies' chunks
                span = (u * nchunks) // 4
                with tc.For_i(
                    0, loop_iters // u, 1,
                    hint_engines=hints, staggered_reset=stag,
                ):
                    for j in range(u):
                        if stag:
                            marks = {
                                (s * span) - j * nchunks
                                for s in range(1, 4)
                                if 0 <= (s * span) - j * nchunks < nchunks
                            }
                        else:
                            marks = ()
                        body(marks)

    nc.finalize()
    _cached_nc[key] = nc
    return nc


_IDENT = None


def _run(x_slab, ca_slab, loop_iters=None, variant="full", **run_kwargs):
    """x_slab (M, N*L) f32, ca_slab (M, N) f32 -> (M, ONC) f32."""
    from concourse.bass_utils import run_bass_kernel_spmd

    global _IDENT
    if _IDENT is None:
        _IDENT = np.eye(R, dtype=np.float32)

    nc = _build_nc(loop_iters, variant)
    in_maps = []
    for c in range(NCORES):
        in_maps.append(
            {
                "x": np.ascontiguousarray(x_slab[c * R : (c + 1) * R]),
                "ca": np.ascontiguousarray(ca_slab[c * R : (c + 1) * R]),
                "ident": _IDENT,
            }
        )
    res = run_bass_kernel_spmd(nc, in_maps, core_ids=list(range(NCORES)), **run_kwargs)
    out = np.concatenate(
        [np.asarray(res.results[c]["out"]) for c in range(NCORES)], axis=0
    )
    return out, res


def kernel(x, ca):
    x = np.ascontiguousarray(np.asarray(x, dtype=np.float32).reshape(M, N * L))
    ca = np.ascontiguousarray(np.asarray(ca, dtype=np.float32).reshape(M, N))
    out, _ = _run(x, ca)
    return out.reshape(1, M, ONC, 1)
